# revision 2
# baseline (speedup 1.0000x reference)
# Trainium2 Bass kernel for an 8-expert top-2 MoE layer (B=4, S=2048, D=1024,
# H=4096), expert-parallel across 8 NeuronCores (one expert per core).
#
# Per core (SPMD, same program; per-core inputs select the expert):
#   Phase R: fp32 gating logits for all 8192 tokens (x^T streamed from HBM),
#     top-2 via the DVE max8/max_index ops, softmax-of-2 via sigmoid, then
#     compaction of the tokens routed to this core's expert: per-partition
#     prefix scan + strict-triangular matmul for the cross-partition prefix,
#     and per-chunk indirect scatters of (token_id, combine_weight) pairs
#     into a compact DRAM list. Unrouted tokens go to a trash slot.
#   Phase F: for 5 static 512-token blocks: indirect-gather the routed token
#     rows (bf16), transpose on the PE, then the expert FFN
#     gelu(x@W1+b1)@W2+b2 in bf16 with fp32 PSUM accumulation (b2 added via a
#     K=1 matmul), scale rows by the combine weight, and indirect-scatter into
#     this core's partial output. Pad slots carry weight 0 and target a trash
#     row, so they contribute nothing.
# Host: shards/replicates inputs, sums the 8 per-core partial outputs
# (each token appears on exactly its two routed cores; other rows stay zero).
import numpy as np
import ml_dtypes

import concourse.bass as bass
import concourse.bacc as bacc
import concourse.mybir as mybir
import concourse.tile as tile
from concourse.bass_utils import run_bass_kernel_spmd
from concourse.masks import make_identity

dt = mybir.dt
AF = mybir.ActivationFunctionType
OP = mybir.AluOpType
P = 128

B, S, D, H, E = 4, 2048, 1024, 4096, 8
N = B * S
NX = N + 1
TB = 512            # FFN token block
NBLK = 5            # blocks per expert (capacity 2560 >= observed max ~2200)
CCAP = TB * NBLK
NCORES = 8


def _build():
    DC = D // P
    HC = H // P
    NCHUNK = N // P
    TOKG = 512
    NG = N // TOKG
    CHG = TOKG // P
    OC = min(512, D)
    NOC = D // OC
    SUBS = TB // P

    nc = bacc.Bacc("TRN2", target_bir_lowering=False, debug=False,
                   num_devices=NCORES)
    xt_d = nc.dram_tensor("xt", [D, N], dt.float32, kind="ExternalInput")
    xb_d = nc.dram_tensor("xb", [NX, D], dt.bfloat16, kind="ExternalInput")
    wg_d = nc.dram_tensor("wg", [D, E], dt.float32, kind="ExternalInput")
    bg_d = nc.dram_tensor("bg", [E, 1], dt.float32, kind="ExternalInput")
    w1_d = nc.dram_tensor("w1", [D, H], dt.bfloat16, kind="ExternalInput")
    b1_d = nc.dram_tensor("b1", [H], dt.float32, kind="ExternalInput")
    w2_d = nc.dram_tensor("w2", [H, D], dt.bfloat16, kind="ExternalInput")
    b2_d = nc.dram_tensor("b2", [1, D], dt.bfloat16, kind="ExternalInput")
    eid_d = nc.dram_tensor("eid", [P, 1], dt.float32, kind="ExternalInput")
    y_d = nc.dram_tensor("y", [NX, D], dt.float32, kind="ExternalOutput")
    logits_d = nc.dram_tensor("logits", [N, E], dt.float32, kind="ExternalOutput")
    topidx_d = nc.dram_tensor("topidx", [N, 2], dt.int32, kind="ExternalOutput")
    idw_d = nc.dram_tensor("idw", [CCAP + 1, 2], dt.float32, kind="Internal")

    with tile.TileContext(nc) as tc:
        with tc.tile_pool(name="const", bufs=1) as cpool, \
             tc.tile_pool(name="ps", bufs=4, space="PSUM") as pspool, \
             tc.tile_pool(name="pst", bufs=2, space="PSUM") as pstpool:
            idn_bf = cpool.tile([P, P], dt.bfloat16, name="idn_bf")
            make_identity(nc, idn_bf[:])
            idn8 = cpool.tile([E, E], dt.float32, name="idn8")
            make_identity(nc, idn8[:])
            U = cpool.tile([P, P], dt.float32, name="U")
            nc.gpsimd.memset(U[:], 0.0)
            nc.gpsimd.affine_select(out=U[:], in_=U[:], compare_op=OP.is_ge,
                                    fill=1.0, base=0, pattern=[[-1, P]],
                                    channel_multiplier=1)
            ones_bf = cpool.tile([1, P], dt.bfloat16, name="ones_bf")
            nc.vector.memset(ones_bf[:], 1.0)
            w1sb = cpool.tile([P, DC * H], dt.bfloat16, name="w1sb")
            for d in range(DC):
                nc.sync.dma_start(out=w1sb[:, d * H:(d + 1) * H],
                                  in_=w1_d[d * P:(d + 1) * P, :])
            w2sb = cpool.tile([P, HC * D], dt.bfloat16, name="w2sb")
            for h in range(HC):
                nc.sync.dma_start(out=w2sb[:, h * D:(h + 1) * D],
                                  in_=w2_d[h * P:(h + 1) * P, :])
            b1sb = cpool.tile([P, HC], dt.float32, name="b1sb")
            nc.sync.dma_start(out=b1sb[:],
                              in_=b1_d[:].rearrange("(hc p) -> p hc", p=P))
            b2sb = cpool.tile([1, D], dt.bfloat16, name="b2sb")
            nc.sync.dma_start(out=b2sb[:], in_=b2_d[:, :])
            wgsb = cpool.tile([P, DC * E], dt.float32, name="wgsb")
            nc.sync.dma_start(out=wgsb[:].rearrange("p (dc e) -> p dc e", e=E),
                              in_=wg_d[:, :].rearrange("(dc p) e -> p dc e", p=P))
            bgsb = cpool.tile([E, 1], dt.float32, name="bgsb")
            nc.sync.dma_start(out=bgsb[:], in_=bg_d[:, :])
            eid = cpool.tile([P, 1], dt.float32, name="eid")
            nc.sync.dma_start(out=eid[:], in_=eid_d[:, :])
            idwinit = cpool.tile([P, 2], dt.float32, name="idwinit")
            nc.vector.memset(idwinit[:, 0:1], float(N))
            nc.vector.memset(idwinit[:, 1:2], 0.0)
            r = 0
            while r < CCAP + 1:
                take = min(P, CCAP + 1 - r)
                nc.sync.dma_start(out=idw_d[r:r + take, :], in_=idwinit[:take, :])
                r += take

            # ---- Phase R ----
            rstack = tc.tile_pool(name="rpool", bufs=1)
            rpool = rstack.__enter__()
            rwstack = tc.tile_pool(name="rwork", bufs=2)
            rwork = rwstack.__enter__()
            logits_sb = rpool.tile([P, NCHUNK * E], dt.float32, name="logits_sb")
            for g in range(NG):
                psg = pspool.tile([E, TOKG], dt.float32, name="psg", tag="big")
                for d in range(DC):
                    xtt = rwork.tile([P, TOKG], dt.float32, name="xtt", tag="xtt")
                    nc.sync.dma_start(
                        out=xtt[:],
                        in_=xt_d[d * P:(d + 1) * P, g * TOKG:(g + 1) * TOKG])
                    nc.tensor.matmul(psg[:], lhsT=wgsb[:, d * E:(d + 1) * E],
                                     rhs=xtt[:], start=(d == 0),
                                     stop=(d == DC - 1))
                lsb8 = rwork.tile([E, TOKG], dt.float32, name="lsb8", tag="lsb8")
                nc.vector.tensor_scalar(out=lsb8[:], in0=psg[:],
                                        scalar1=bgsb[:, 0:1], scalar2=None,
                                        op0=OP.add)
                for i in range(CHG):
                    c = g * CHG + i
                    pstf = pstpool.tile([P, E], dt.float32, name="pstf", tag="trf")
                    nc.tensor.transpose(pstf[:], in_=lsb8[:, i * P:(i + 1) * P],
                                        identity=idn8[:])
                    nc.vector.tensor_copy(out=logits_sb[:, c * E:(c + 1) * E],
                                          in_=pstf[:])

            maxv = rpool.tile([P, NCHUNK * E], dt.float32, name="maxv")
            idxv = rpool.tile([P, NCHUNK * E], dt.uint32, name="idxv")
            for c in range(NCHUNK):
                s = slice(c * E, (c + 1) * E)
                nc.vector.max(out=maxv[:, s], in_=logits_sb[:, s])
                nc.vector.max_index(out=idxv[:, s], in_max=maxv[:, s],
                                    in_values=logits_sb[:, s])
            maxv3 = maxv[:].rearrange("p (c k) -> p c k", k=E)
            idx3 = idxv[:].rearrange("p (c k) -> p c k", k=E)

            def rt(name, d=dt.float32):
                return rpool.tile([P, NCHUNK], d, name=name)

            w2t, w1t = rt("w2t"), rt("w1t")
            i1f, i2f = rt("i1f"), rt("i2f")
            eq1, eq2 = rt("eq1"), rt("eq2")
            maskt, wct, tmpt = rt("maskt"), rt("wct"), rt("tmpt")
            zeros, scant, post = rt("zeros"), rt("scant"), rt("post")
            posi = rt("posi", dt.int32)
            idsi = rt("idsi", dt.int32)
            idsf = rt("idsf")
            poff = rpool.tile([P, 1], dt.float32, name="poff")
            nc.vector.tensor_tensor(out=w2t[:], in0=maxv3[:, :, 1],
                                    in1=maxv3[:, :, 0], op=OP.subtract)
            nc.scalar.activation(out=w2t[:], in_=w2t[:], func=AF.Sigmoid)
            nc.vector.tensor_scalar(out=w1t[:], in0=w2t[:], scalar1=-1.0,
                                    scalar2=-1.0, op0=OP.mult, op1=OP.subtract)
            nc.vector.tensor_copy(out=i1f[:], in_=idx3[:, :, 0])
            nc.vector.tensor_copy(out=i2f[:], in_=idx3[:, :, 1])
            nc.vector.tensor_scalar(out=eq1[:], in0=i1f[:], scalar1=eid[:, 0:1],
                                    scalar2=None, op0=OP.is_equal)
            nc.vector.tensor_scalar(out=eq2[:], in0=i2f[:], scalar1=eid[:, 0:1],
                                    scalar2=None, op0=OP.is_equal)
            nc.vector.tensor_tensor(out=maskt[:], in0=eq1[:], in1=eq2[:], op=OP.add)
            nc.vector.tensor_tensor(out=wct[:], in0=eq1[:], in1=w1t[:], op=OP.mult)
            nc.vector.tensor_tensor(out=tmpt[:], in0=eq2[:], in1=w2t[:], op=OP.mult)
            nc.vector.tensor_tensor(out=wct[:], in0=wct[:], in1=tmpt[:], op=OP.add)
            nc.vector.memset(zeros[:], 0.0)
            nc.vector.tensor_tensor_scan(out=scant[:], data0=maskt[:],
                                         data1=zeros[:], initial=0.0,
                                         op0=OP.add, op1=OP.add)
            poffp = pspool.tile([P, 1], dt.float32, name="poffp", tag="big")
            nc.tensor.matmul(poffp[:], lhsT=U[:],
                             rhs=scant[:, NCHUNK - 1:NCHUNK],
                             start=True, stop=True)
            nc.vector.tensor_copy(out=poff[:], in_=poffp[:])
            nc.vector.tensor_tensor(out=post[:], in0=scant[:], in1=maskt[:],
                                    op=OP.subtract)
            nc.vector.tensor_scalar(out=post[:], in0=post[:],
                                    scalar1=poff[:, 0:1], scalar2=None, op0=OP.add)
            nc.vector.tensor_scalar(out=tmpt[:], in0=maskt[:],
                                    scalar1=float(-CCAP), scalar2=float(CCAP),
                                    op0=OP.mult, op1=OP.add)
            nc.vector.tensor_tensor(out=post[:], in0=post[:], in1=tmpt[:], op=OP.add)
            nc.vector.tensor_scalar_min(post[:], post[:], float(CCAP))
            nc.vector.tensor_copy(out=posi[:], in_=post[:])
            nc.gpsimd.iota(idsi[:], pattern=[[P, NCHUNK]], base=0,
                           channel_multiplier=1)
            nc.vector.tensor_copy(out=idsf[:], in_=idsi[:])
            idw_pair = rpool.tile([P, 2 * NCHUNK], dt.float32, name="idw_pair")
            idwp3 = idw_pair[:].rearrange("p (c k) -> p c k", k=2)
            nc.vector.tensor_copy(out=idwp3[:, :, 0], in_=idsf[:])
            nc.vector.tensor_copy(out=idwp3[:, :, 1], in_=wct[:])
            for c in range(NCHUNK):
                nc.gpsimd.indirect_dma_start(
                    out=idw_d[:, :],
                    out_offset=bass.IndirectOffsetOnAxis(ap=posi[:, c:c + 1], axis=0),
                    in_=idw_pair[:, 2 * c:2 * c + 2],
                    in_offset=None)
            idxi = rpool.tile([P, 2 * NCHUNK], dt.int32, name="idxi")
            idxi3 = idxi[:].rearrange("p (c k) -> p c k", k=2)
            nc.vector.tensor_copy(out=idxi3[:, :, 0], in_=idx3[:, :, 0])
            nc.vector.tensor_copy(out=idxi3[:, :, 1], in_=idx3[:, :, 1])
            nc.sync.dma_start(
                out=logits_d[:, :].rearrange("(c p) e -> p c e", p=P),
                in_=logits_sb[:].rearrange("p (c e) -> p c e", e=E))
            nc.sync.dma_start(
                out=topidx_d[:, :].rearrange("(c p) k -> p c k", p=P),
                in_=idxi[:].rearrange("p (c k) -> p c k", k=2))

            rwstack.__exit__(None, None, None)
            rstack.__exit__(None, None, None)

            # ---- Phase F ----
            fstack = tc.tile_pool(name="fpool", bufs=1)
            fpool = fstack.__enter__()
            fwstack = tc.tile_pool(name="fwork", bufs=2)
            fwork = fwstack.__enter__()
            fsstack = tc.tile_pool(name="fsmall", bufs=3 * SUBS)
            spool = fsstack.__enter__()
            for blk in range(NBLK):
                xgT = fpool.tile([P, DC * TB], dt.bfloat16, name="xgT", tag="xgT")
                hT = fpool.tile([P, HC * TB], dt.bfloat16, name="hT", tag="hT")
                ids_l, w_l = [], []
                for sub in range(SUBS):
                    idwt = spool.tile([P, 2], dt.float32, name="idwt", tag="idwt")
                    base = blk * TB + sub * P
                    nc.sync.dma_start(out=idwt[:], in_=idw_d[base:base + P, :])
                    gidi = spool.tile([P, 1], dt.int32, name="gidi", tag="gidi")
                    nc.vector.tensor_copy(out=gidi[:], in_=idwt[:, 0:1])
                    gw = spool.tile([P, 1], dt.float32, name="gw", tag="gw")
                    nc.vector.tensor_copy(out=gw[:], in_=idwt[:, 1:2])
                    xrow = fwork.tile([P, D], dt.bfloat16, name="xrow", tag="xrow")
                    nc.gpsimd.indirect_dma_start(
                        out=xrow[:], out_offset=None, in_=xb_d[:, :],
                        in_offset=bass.IndirectOffsetOnAxis(ap=gidi[:], axis=0))
                    for d2 in range(DC):
                        pstb = pstpool.tile([P, P], dt.bfloat16, name="pstb",
                                            tag="trb")
                        nc.tensor.transpose(pstb[:], in_=xrow[:, d2 * P:(d2 + 1) * P],
                                            identity=idn_bf[:])
                        nc.vector.tensor_copy(
                            out=xgT[:, d2 * TB + sub * P: d2 * TB + (sub + 1) * P],
                            in_=pstb[:])
                    ids_l.append(gidi)
                    w_l.append(gw)
                for h in range(HC):
                    ps1 = pspool.tile([P, TB], dt.float32, name="ps1", tag="big")
                    for d2 in range(DC):
                        nc.tensor.matmul(
                            ps1[:],
                            lhsT=w1sb[:, d2 * H + h * P: d2 * H + (h + 1) * P],
                            rhs=xgT[:, d2 * TB:(d2 + 1) * TB],
                            start=(d2 == 0), stop=(d2 == DC - 1))
                    nc.scalar.activation(out=hT[:, h * TB:(h + 1) * TB],
                                         in_=ps1[:], func=AF.Gelu,
                                         bias=b1sb[:, h:h + 1], scale=1.0)
                for sub in range(SUBS):
                    ysb = fwork.tile([P, D], dt.float32, name="ysb", tag="ysb")
                    for oc in range(NOC):
                        ps2 = pspool.tile([P, OC], dt.float32, name="ps2", tag="big")
                        for h in range(HC):
                            nc.tensor.matmul(
                                ps2[:],
                                lhsT=hT[:, h * TB + sub * P: h * TB + (sub + 1) * P],
                                rhs=w2sb[:, h * D + oc * OC: h * D + (oc + 1) * OC],
                                start=(h == 0), stop=False)
                        nc.tensor.matmul(ps2[:], lhsT=ones_bf[:],
                                         rhs=b2sb[:, oc * OC:(oc + 1) * OC],
                                         start=False, stop=True)
                        nc.vector.tensor_scalar(out=ysb[:, oc * OC:(oc + 1) * OC],
                                                in0=ps2[:],
                                                scalar1=w_l[sub][:, 0:1],
                                                scalar2=None, op0=OP.mult)
                    nc.gpsimd.indirect_dma_start(
                        out=y_d[:, :],
                        out_offset=bass.IndirectOffsetOnAxis(ap=ids_l[sub][:], axis=0),
                        in_=ysb[:], in_offset=None)
            fsstack.__exit__(None, None, None)
            fwstack.__exit__(None, None, None)
            fstack.__exit__(None, None, None)
    nc.compile()
    return nc


_NC_CACHE = None


def _get_nc():
    global _NC_CACHE
    if _NC_CACHE is None:
        _NC_CACHE = _build()
    return _NC_CACHE


def _make_in_maps(x, Wg, bg, W1, b1, W2, b2):
    bf = ml_dtypes.bfloat16
    xf = np.ascontiguousarray(np.asarray(x, dtype=np.float32).reshape(N, D))
    xt = np.ascontiguousarray(xf.T)
    xb = np.zeros((NX, D), dtype=bf)
    xb[:N] = xf.astype(bf)
    wg = np.ascontiguousarray(np.asarray(Wg, dtype=np.float32))
    bgc = np.ascontiguousarray(np.asarray(bg, dtype=np.float32).reshape(E, 1))
    maps = []
    for c in range(NCORES):
        maps.append({
            "xt": xt, "xb": xb, "wg": wg, "bg": bgc,
            "w1": np.ascontiguousarray(np.asarray(W1[c], np.float32)).astype(bf),
            "b1": np.ascontiguousarray(np.asarray(b1[c], np.float32)),
            "w2": np.ascontiguousarray(np.asarray(W2[c], np.float32)).astype(bf),
            "b2": np.asarray(b2[c], np.float32).reshape(1, D).astype(bf),
            "eid": np.full((P, 1), float(c), np.float32),
        })
    return maps


def run(x, Wg, bg, W1, b1, W2, b2, trace=False, **spmd_kwargs):
    nc = _get_nc()
    in_maps = _make_in_maps(x, Wg, bg, W1, b1, W2, b2)
    res = run_bass_kernel_spmd(nc, in_maps, core_ids=list(range(NCORES)),
                               trace=trace, **spmd_kwargs)
    y = np.zeros((N, D), np.float32)
    for c in range(NCORES):
        y += res.results[c]["y"][:N]
    out = y.reshape(B, S, D)
    logits = res.results[0]["logits"].reshape(B, S, E).astype(np.float32)
    topidx = res.results[0]["topidx"].reshape(B, S, 2).astype(np.int32)
    return (out, {"gating_logits": logits, "top_indices": topidx}), res


def kernel(x, Wg, bg, W1, b1, W2, b2):
    out, _ = run(x, Wg, bg, W1, b1, W2, b2, trace=False)
    return out


# revision 3
# speedup vs baseline: 1.4509x; 1.4509x over previous
# Trainium2 Bass kernel for an 8-expert top-2 MoE layer (B=4, S=2048, D=1024,
# H=4096), expert-parallel across 8 NeuronCores (one expert per core).
#
# Per core (SPMD, same program; per-core inputs select the expert):
#   Phase R: fp32 gating logits for all 8192 tokens (x^T streamed from HBM),
#     top-2 via the DVE max8/max_index ops, softmax-of-2 via sigmoid, then
#     compaction of the tokens routed to this core's expert: per-partition
#     prefix scan + strict-triangular matmul for the cross-partition prefix,
#     and per-chunk indirect scatters of (token_id, combine_weight) pairs
#     into a compact DRAM list. Unrouted tokens go to a trash slot.
#   Phase F: for 5 static 512-token blocks: indirect-gather the routed token
#     rows (bf16), transpose on the PE, then the expert FFN
#     gelu(x@W1+b1)@W2+b2 in bf16 with fp32 PSUM accumulation (b2 added via a
#     K=1 matmul), scale rows by the combine weight, and indirect-scatter into
#     this core's partial output. Pad slots carry weight 0 and target a trash
#     row, so they contribute nothing.
# Host: shards/replicates inputs, sums the 8 per-core partial outputs
# (each token appears on exactly its two routed cores; other rows stay zero).
import numpy as np
import ml_dtypes

import concourse.bass as bass
import concourse.bacc as bacc
import concourse.mybir as mybir
import concourse.tile as tile
from concourse.bass_utils import run_bass_kernel_spmd
from concourse.masks import make_identity

dt = mybir.dt
AF = mybir.ActivationFunctionType
OP = mybir.AluOpType
P = 128

B, S, D, H, E = 4, 2048, 1024, 4096, 8
N = B * S
NX = N + 1
TB = 512            # FFN token block
NBLK = 5            # blocks per expert (capacity 2560 >= observed max ~2200)
CCAP = TB * NBLK
NCORES = 8


def _build():
    DC = D // P
    HC = H // P
    NCHUNK = N // P
    TOKG = 512
    NG = N // TOKG
    CHG = TOKG // P
    OC = min(512, D)
    NOC = D // OC
    SUBS = TB // P

    nc = bacc.Bacc("TRN2", target_bir_lowering=False, debug=False,
                   num_devices=NCORES)
    xt_d = nc.dram_tensor("xt", [D, N], dt.float32, kind="ExternalInput")
    xb_d = nc.dram_tensor("xb", [NX, D], dt.bfloat16, kind="ExternalInput")
    wg_d = nc.dram_tensor("wg", [D, E], dt.float32, kind="ExternalInput")
    bg_d = nc.dram_tensor("bg", [E, 1], dt.float32, kind="ExternalInput")
    w1_d = nc.dram_tensor("w1", [D, H], dt.bfloat16, kind="ExternalInput")
    b1_d = nc.dram_tensor("b1", [H], dt.float32, kind="ExternalInput")
    w2_d = nc.dram_tensor("w2", [H, D], dt.bfloat16, kind="ExternalInput")
    b2_d = nc.dram_tensor("b2", [1, D], dt.bfloat16, kind="ExternalInput")
    eid_d = nc.dram_tensor("eid", [P, 1], dt.float32, kind="ExternalInput")
    y_d = nc.dram_tensor("y", [NX, D], dt.float32, kind="ExternalOutput")
    logits_d = nc.dram_tensor("logits", [N, E], dt.float32, kind="ExternalOutput")
    topidx_d = nc.dram_tensor("topidx", [N, 2], dt.int32, kind="ExternalOutput")
    QSP = 8             # compaction scatter split (independent WAW chains)
    IDWR = ((CCAP + 1 + P - 1) // P) * P
    idw_ds = [nc.dram_tensor(f"idw{q}", [IDWR, 2], dt.float32, kind="Internal")
              for q in range(QSP)]

    with tile.TileContext(nc) as tc:
        with tc.tile_pool(name="const", bufs=1) as cpool, \
             tc.tile_pool(name="ps", bufs=4, space="PSUM") as pspool, \
             tc.tile_pool(name="pst", bufs=2, space="PSUM") as pstpool:
            idn_bf = cpool.tile([P, P], dt.bfloat16, name="idn_bf")
            make_identity(nc, idn_bf[:])
            idn8 = cpool.tile([E, E], dt.float32, name="idn8")
            make_identity(nc, idn8[:])
            U = cpool.tile([P, P], dt.float32, name="U")
            nc.gpsimd.memset(U[:], 0.0)
            nc.gpsimd.affine_select(out=U[:], in_=U[:], compare_op=OP.is_ge,
                                    fill=1.0, base=0, pattern=[[-1, P]],
                                    channel_multiplier=1)
            ones_bf = cpool.tile([1, P], dt.bfloat16, name="ones_bf")
            nc.vector.memset(ones_bf[:], 1.0)
            w1sb = cpool.tile([P, DC * H], dt.bfloat16, name="w1sb")
            w2sb = cpool.tile([P, HC * D], dt.bfloat16, name="w2sb")
            b1sb = cpool.tile([P, HC], dt.float32, name="b1sb")
            b2sb = cpool.tile([1, D], dt.bfloat16, name="b2sb")

            def load_weights():
                for d in range(DC):
                    nc.gpsimd.dma_start(out=w1sb[:, d * H:(d + 1) * H],
                                        in_=w1_d[d * P:(d + 1) * P, :])
                for h in range(HC):
                    nc.gpsimd.dma_start(out=w2sb[:, h * D:(h + 1) * D],
                                        in_=w2_d[h * P:(h + 1) * P, :])
                nc.gpsimd.dma_start(out=b1sb[:],
                                    in_=b1_d[:].rearrange("(hc p) -> p hc", p=P))
                nc.gpsimd.dma_start(out=b2sb[:], in_=b2_d[:, :])
            wgsb = cpool.tile([P, DC * E], dt.float32, name="wgsb")
            nc.sync.dma_start(out=wgsb[:].rearrange("p (dc e) -> p dc e", e=E),
                              in_=wg_d[:, :].rearrange("(dc p) e -> p dc e", p=P))
            bgsb = cpool.tile([E, 1], dt.float32, name="bgsb")
            nc.sync.dma_start(out=bgsb[:], in_=bg_d[:, :])
            eid = cpool.tile([P, 1], dt.float32, name="eid")
            nc.sync.dma_start(out=eid[:], in_=eid_d[:, :])
            NG_I = IDWR // P
            idwinit = cpool.tile([P, 2 * NG_I], dt.float32, name="idwinit")
            idwinit3 = idwinit[:].rearrange("p (g k) -> p g k", k=2)
            nc.vector.memset(idwinit3[:, :, 0], float(N))
            nc.vector.memset(idwinit3[:, :, 1], 0.0)
            for q in range(QSP):
                nc.sync.dma_start(
                    out=idw_ds[q][:, :].rearrange("(g p) k -> p g k", p=P),
                    in_=idwinit3)

            # ---- Phase R ----
            rstack = tc.tile_pool(name="rpool", bufs=1)
            rpool = rstack.__enter__()
            rwstack = tc.tile_pool(name="rwork", bufs=4)
            rwork = rwstack.__enter__()
            logits_sb = rpool.tile([P, NCHUNK * E], dt.float32, name="logits_sb")
            for g in range(NG):
                psg = pspool.tile([E, TOKG], dt.float32, name="psg", tag="big")
                for d in range(DC):
                    xtt = rwork.tile([P, TOKG], dt.float32, name="xtt", tag="xtt")
                    nc.sync.dma_start(
                        out=xtt[:],
                        in_=xt_d[d * P:(d + 1) * P, g * TOKG:(g + 1) * TOKG])
                    nc.tensor.matmul(psg[:], lhsT=wgsb[:, d * E:(d + 1) * E],
                                     rhs=xtt[:], start=(d == 0),
                                     stop=(d == DC - 1))
                lsb8 = rwork.tile([E, TOKG], dt.float32, name="lsb8", tag="lsb8")
                nc.vector.tensor_scalar(out=lsb8[:], in0=psg[:],
                                        scalar1=bgsb[:, 0:1], scalar2=None,
                                        op0=OP.add)
                for i in range(CHG):
                    c = g * CHG + i
                    pstf = pstpool.tile([P, E], dt.float32, name="pstf", tag="trf")
                    nc.tensor.transpose(pstf[:], in_=lsb8[:, i * P:(i + 1) * P],
                                        identity=idn8[:])
                    nc.vector.tensor_copy(out=logits_sb[:, c * E:(c + 1) * E],
                                          in_=pstf[:])

            load_weights()

            maxv = rpool.tile([P, NCHUNK * E], dt.float32, name="maxv")
            idxv = rpool.tile([P, NCHUNK * E], dt.uint32, name="idxv")
            for c in range(NCHUNK):
                s = slice(c * E, (c + 1) * E)
                nc.vector.max(out=maxv[:, s], in_=logits_sb[:, s])
                nc.vector.max_index(out=idxv[:, s], in_max=maxv[:, s],
                                    in_values=logits_sb[:, s])
            maxv3 = maxv[:].rearrange("p (c k) -> p c k", k=E)
            idx3 = idxv[:].rearrange("p (c k) -> p c k", k=E)

            def rt(name, d=dt.float32):
                return rpool.tile([P, NCHUNK], d, name=name)

            w2t, w1t = rt("w2t"), rt("w1t")
            i1f, i2f = rt("i1f"), rt("i2f")
            eq1, eq2 = rt("eq1"), rt("eq2")
            maskt, wct, tmpt = rt("maskt"), rt("wct"), rt("tmpt")
            zeros, scant, post = rt("zeros"), rt("scant"), rt("post")
            posi = rt("posi", dt.int32)
            idsi = rt("idsi", dt.int32)
            idsf = rt("idsf")
            poff = rpool.tile([P, 1], dt.float32, name="poff")
            nc.vector.tensor_tensor(out=w2t[:], in0=maxv3[:, :, 1],
                                    in1=maxv3[:, :, 0], op=OP.subtract)
            nc.scalar.activation(out=w2t[:], in_=w2t[:], func=AF.Sigmoid)
            nc.vector.tensor_scalar(out=w1t[:], in0=w2t[:], scalar1=-1.0,
                                    scalar2=-1.0, op0=OP.mult, op1=OP.subtract)
            nc.vector.tensor_copy(out=i1f[:], in_=idx3[:, :, 0])
            nc.vector.tensor_copy(out=i2f[:], in_=idx3[:, :, 1])
            nc.vector.tensor_scalar(out=eq1[:], in0=i1f[:], scalar1=eid[:, 0:1],
                                    scalar2=None, op0=OP.is_equal)
            nc.vector.tensor_scalar(out=eq2[:], in0=i2f[:], scalar1=eid[:, 0:1],
                                    scalar2=None, op0=OP.is_equal)
            nc.vector.tensor_tensor(out=maskt[:], in0=eq1[:], in1=eq2[:], op=OP.add)
            nc.vector.tensor_tensor(out=wct[:], in0=eq1[:], in1=w1t[:], op=OP.mult)
            nc.vector.tensor_tensor(out=tmpt[:], in0=eq2[:], in1=w2t[:], op=OP.mult)
            nc.vector.tensor_tensor(out=wct[:], in0=wct[:], in1=tmpt[:], op=OP.add)
            nc.vector.memset(zeros[:], 0.0)
            nc.vector.tensor_tensor_scan(out=scant[:], data0=maskt[:],
                                         data1=zeros[:], initial=0.0,
                                         op0=OP.add, op1=OP.add)
            poffp = pspool.tile([P, 1], dt.float32, name="poffp", tag="big")
            nc.tensor.matmul(poffp[:], lhsT=U[:],
                             rhs=scant[:, NCHUNK - 1:NCHUNK],
                             start=True, stop=True)
            nc.vector.tensor_copy(out=poff[:], in_=poffp[:])
            nc.vector.tensor_tensor(out=post[:], in0=scant[:], in1=maskt[:],
                                    op=OP.subtract)
            nc.vector.tensor_scalar(out=post[:], in0=post[:],
                                    scalar1=poff[:, 0:1], scalar2=None, op0=OP.add)
            nc.vector.tensor_scalar(out=tmpt[:], in0=maskt[:],
                                    scalar1=float(-CCAP), scalar2=float(CCAP),
                                    op0=OP.mult, op1=OP.add)
            nc.vector.tensor_tensor(out=post[:], in0=post[:], in1=tmpt[:], op=OP.add)
            nc.vector.tensor_scalar_min(post[:], post[:], float(CCAP))
            nc.vector.tensor_copy(out=posi[:], in_=post[:])
            nc.gpsimd.iota(idsi[:], pattern=[[P, NCHUNK]], base=0,
                           channel_multiplier=1)
            nc.vector.tensor_copy(out=idsf[:], in_=idsi[:])
            idw_pair = rpool.tile([P, 2 * NCHUNK], dt.float32, name="idw_pair")
            idwp3 = idw_pair[:].rearrange("p (c k) -> p c k", k=2)
            nc.vector.tensor_copy(out=idwp3[:, :, 0], in_=idsf[:])
            nc.vector.tensor_copy(out=idwp3[:, :, 1], in_=wct[:])
            for c in range(NCHUNK):
                nc.gpsimd.indirect_dma_start(
                    out=idw_ds[c % QSP][:, :],
                    out_offset=bass.IndirectOffsetOnAxis(ap=posi[:, c:c + 1], axis=0),
                    in_=idw_pair[:, 2 * c:2 * c + 2],
                    in_offset=None)
            idxi = rpool.tile([P, 2 * NCHUNK], dt.int32, name="idxi")
            idxi3 = idxi[:].rearrange("p (c k) -> p c k", k=2)
            nc.vector.tensor_copy(out=idxi3[:, :, 0], in_=idx3[:, :, 0])
            nc.vector.tensor_copy(out=idxi3[:, :, 1], in_=idx3[:, :, 1])
            nc.sync.dma_start(
                out=logits_d[:, :].rearrange("(c p) e -> p c e", p=P),
                in_=logits_sb[:].rearrange("p (c e) -> p c e", e=E))
            nc.sync.dma_start(
                out=topidx_d[:, :].rearrange("(c p) k -> p c k", p=P),
                in_=idxi[:].rearrange("p (c k) -> p c k", k=2))

            rwstack.__exit__(None, None, None)
            rstack.__exit__(None, None, None)

            # ---- Phase F ----
            fstack = tc.tile_pool(name="fpool", bufs=1)
            fpool = fstack.__enter__()
            fwstack = tc.tile_pool(name="fwork", bufs=2)
            fwork = fwstack.__enter__()
            fsstack = tc.tile_pool(name="fsmall", bufs=3 * SUBS)
            spool = fsstack.__enter__()
            for blk in range(NBLK):
                xgT = fpool.tile([P, DC * TB], dt.bfloat16, name="xgT", tag="xgT")
                hT = fpool.tile([P, HC * TB], dt.bfloat16, name="hT", tag="hT")
                ids_l, w_l = [], []
                for sub in range(SUBS):
                    idwt = spool.tile([P, 2 * QSP], dt.float32, name="idwt", tag="idwt")
                    base = blk * TB + sub * P
                    for q in range(QSP):
                        nc.sync.dma_start(out=idwt[:, 2 * q:2 * q + 2],
                                          in_=idw_ds[q][base:base + P, :])
                    idwm = spool.tile([P, 2], dt.float32, name="idwm", tag="idwm")
                    idwt3 = idwt[:].rearrange("p (q k) -> p q k", k=2)
                    # id = min over q (pads are N), w = max over q (pads are 0)
                    nc.vector.tensor_tensor(out=idwm[:, 0:1], in0=idwt3[:, 0, 0:1],
                                            in1=idwt3[:, 1, 0:1], op=OP.min)
                    nc.vector.tensor_tensor(out=idwm[:, 1:2], in0=idwt3[:, 0, 1:2],
                                            in1=idwt3[:, 1, 1:2], op=OP.max)
                    for q in range(2, QSP):
                        nc.vector.tensor_tensor(out=idwm[:, 0:1], in0=idwm[:, 0:1],
                                                in1=idwt3[:, q, 0:1], op=OP.min)
                        nc.vector.tensor_tensor(out=idwm[:, 1:2], in0=idwm[:, 1:2],
                                                in1=idwt3[:, q, 1:2], op=OP.max)
                    gidi = spool.tile([P, 1], dt.int32, name="gidi", tag="gidi")
                    nc.vector.tensor_copy(out=gidi[:], in_=idwm[:, 0:1])
                    gw = spool.tile([P, 1], dt.float32, name="gw", tag="gw")
                    nc.vector.tensor_copy(out=gw[:], in_=idwm[:, 1:2])
                    xrow = fwork.tile([P, D], dt.bfloat16, name="xrow", tag="xrow")
                    nc.gpsimd.indirect_dma_start(
                        out=xrow[:], out_offset=None, in_=xb_d[:, :],
                        in_offset=bass.IndirectOffsetOnAxis(ap=gidi[:], axis=0))
                    for d2 in range(DC):
                        pstb = pstpool.tile([P, P], dt.bfloat16, name="pstb",
                                            tag="trb")
                        nc.tensor.transpose(pstb[:], in_=xrow[:, d2 * P:(d2 + 1) * P],
                                            identity=idn_bf[:])
                        nc.vector.tensor_copy(
                            out=xgT[:, d2 * TB + sub * P: d2 * TB + (sub + 1) * P],
                            in_=pstb[:])
                    ids_l.append(gidi)
                    w_l.append(gw)
                for h in range(HC):
                    ps1 = pspool.tile([P, TB], dt.float32, name="ps1", tag="big")
                    for d2 in range(DC):
                        nc.tensor.matmul(
                            ps1[:],
                            lhsT=w1sb[:, d2 * H + h * P: d2 * H + (h + 1) * P],
                            rhs=xgT[:, d2 * TB:(d2 + 1) * TB],
                            start=(d2 == 0), stop=(d2 == DC - 1))
                    nc.scalar.activation(out=hT[:, h * TB:(h + 1) * TB],
                                         in_=ps1[:], func=AF.Gelu,
                                         bias=b1sb[:, h:h + 1], scale=1.0)
                for sub in range(SUBS):
                    ysb = fwork.tile([P, D], dt.float32, name="ysb", tag="ysb")
                    for oc in range(NOC):
                        ps2 = pspool.tile([P, OC], dt.float32, name="ps2", tag="big")
                        for h in range(HC):
                            nc.tensor.matmul(
                                ps2[:],
                                lhsT=hT[:, h * TB + sub * P: h * TB + (sub + 1) * P],
                                rhs=w2sb[:, h * D + oc * OC: h * D + (oc + 1) * OC],
                                start=(h == 0), stop=False)
                        nc.tensor.matmul(ps2[:], lhsT=ones_bf[:],
                                         rhs=b2sb[:, oc * OC:(oc + 1) * OC],
                                         start=False, stop=True)
                        nc.vector.tensor_scalar(out=ysb[:, oc * OC:(oc + 1) * OC],
                                                in0=ps2[:],
                                                scalar1=w_l[sub][:, 0:1],
                                                scalar2=None, op0=OP.mult)
                    nc.gpsimd.indirect_dma_start(
                        out=y_d[:, :],
                        out_offset=bass.IndirectOffsetOnAxis(ap=ids_l[sub][:], axis=0),
                        in_=ysb[:], in_offset=None)
            fsstack.__exit__(None, None, None)
            fwstack.__exit__(None, None, None)
            fstack.__exit__(None, None, None)
    nc.compile()
    return nc


_NC_CACHE = None


def _get_nc():
    global _NC_CACHE
    if _NC_CACHE is None:
        _NC_CACHE = _build()
    return _NC_CACHE


def _make_in_maps(x, Wg, bg, W1, b1, W2, b2):
    bf = ml_dtypes.bfloat16
    xf = np.ascontiguousarray(np.asarray(x, dtype=np.float32).reshape(N, D))
    xt = np.ascontiguousarray(xf.T)
    xb = np.zeros((NX, D), dtype=bf)
    xb[:N] = xf.astype(bf)
    wg = np.ascontiguousarray(np.asarray(Wg, dtype=np.float32))
    bgc = np.ascontiguousarray(np.asarray(bg, dtype=np.float32).reshape(E, 1))
    maps = []
    for c in range(NCORES):
        maps.append({
            "xt": xt, "xb": xb, "wg": wg, "bg": bgc,
            "w1": np.ascontiguousarray(np.asarray(W1[c], np.float32)).astype(bf),
            "b1": np.ascontiguousarray(np.asarray(b1[c], np.float32)),
            "w2": np.ascontiguousarray(np.asarray(W2[c], np.float32)).astype(bf),
            "b2": np.asarray(b2[c], np.float32).reshape(1, D).astype(bf),
            "eid": np.full((P, 1), float(c), np.float32),
        })
    return maps


def run(x, Wg, bg, W1, b1, W2, b2, trace=False, **spmd_kwargs):
    nc = _get_nc()
    in_maps = _make_in_maps(x, Wg, bg, W1, b1, W2, b2)
    res = run_bass_kernel_spmd(nc, in_maps, core_ids=list(range(NCORES)),
                               trace=trace, **spmd_kwargs)
    y = np.zeros((N, D), np.float32)
    for c in range(NCORES):
        y += res.results[c]["y"][:N]
    out = y.reshape(B, S, D)
    logits = res.results[0]["logits"].reshape(B, S, E).astype(np.float32)
    topidx = res.results[0]["topidx"].reshape(B, S, 2).astype(np.int32)
    return (out, {"gating_logits": logits, "top_indices": topidx}), res


def kernel(x, Wg, bg, W1, b1, W2, b2):
    out, _ = run(x, Wg, bg, W1, b1, W2, b2, trace=False)
    return out
